# revision 2
# baseline (speedup 1.0000x reference)
"""AFT-Full transformer encoder block on 8 Trainium2 NeuronCores.

Sharding: data-parallel over batch (B=8 -> 1 batch element per core), all
weights replicated. No collectives.

Per-core layout strategy ("T-layout"): every on-chip activation that feeds a
matmul keeps its contraction dimension on SBUF partitions, so the kernel
needs zero on-chip transposes:
  - host feeds x[b] transposed:  xT [F=512, T=1024]
  - host feeds w_pos transposed: w_posT [S=1024, T=1024]
  - hT [f,t]: lhsT for K/V (N-layout out [t,f]) and rhs for Q (T-layout out)
  - X = [exp_K*V | exp_K] in N-layout [s, 2F] is the lhsT of num/den
  - exp_wT [s,t] is the rhs of num/den -> numT/denT [f,t] (T-layout)
  - attn/mlp outputs stay T-layout; output yT [F, T] transposed back on host

Matmuls run in bf16 (1 cycle/row, fp32 PSUM accumulation); all layernorm
statistics, elementwise math, exp/sigmoid/gelu inputs, residuals are fp32.
The exp_w row-max subtraction is skipped: it cancels exactly in num/den.
LayerNorm statistics are computed with an all-ones [128,128] stationary
matmul, which yields partition-replicated sums (no partition broadcast
needed).
"""
import functools
import numpy as np
import ml_dtypes

import concourse.bacc as bacc
import concourse.tile as tile
import concourse.mybir as mybir
from concourse.bass_utils import run_bass_kernel_spmd

P = 128
B, T, F, H = 8, 1024, 512, 2048
FT = F // P      # 4 feature tiles
TT = T // P      # 8 token tiles
HT = H // P      # 16 hidden tiles
CH = 512         # token chunk (one PSUM bank of fp32)
NC = T // CH     # 2 chunks
LN_EPS = 1e-5

f32 = mybir.dt.float32
bf16 = mybir.dt.bfloat16
ALU = mybir.AluOpType
AF = mybir.ActivationFunctionType


def _emit_layernorm(nc, tc, psum, ln_tmp, src, srcb, sqb, g_pm, b_pm, ones, out_b):
    """LayerNorm over the partition (feature) axis of T-layout src.

    src  : fp32 [P, FT, T] input
    srcb : bf16 [P, FT, T] copy of src (stats matmul operand)
    sqb  : bf16 [P, FT, T] square of src (stats matmul operand)
    out_b: bf16 [P, FT, T] normalized output
    g_pm, b_pm: fp32 [P, FT] gain/bias, partition-major
    Stats = ones[128,128].T @ srcb -> partition-replicated column sums.
    """
    for c in range(NC):
        ts = slice(c * CH, (c + 1) * CH)
        s1 = psum.tile([P, CH], f32, tag="acc")
        for ft in range(FT):
            nc.tensor.matmul(s1[:], ones[:, :P], srcb[:, ft, ts],
                             start=(ft == 0), stop=(ft == FT - 1))
        s2 = psum.tile([P, CH], f32, tag="acc")
        for ft in range(FT):
            nc.tensor.matmul(s2[:], ones[:, :P], sqb[:, ft, ts],
                             start=(ft == 0), stop=(ft == FT - 1))
        # mean, var, rstd (all partition-replicated [P, CH])
        mval = ln_tmp.tile([P, CH], f32, tag="mval")
        nc.vector.tensor_scalar_mul(mval[:], s1[:], 1.0 / F)
        msq = ln_tmp.tile([P, CH], f32, tag="msq")
        nc.vector.tensor_tensor(msq[:], mval[:], mval[:], op=ALU.mult)
        var = ln_tmp.tile([P, CH], f32, tag="var")
        nc.vector.scalar_tensor_tensor(var[:], s2[:], 1.0 / F, msq[:],
                                       op0=ALU.mult, op1=ALU.subtract)
        varp = ln_tmp.tile([P, CH], f32, tag="varp")
        nc.vector.tensor_scalar_add(varp[:], var[:], LN_EPS)
        rcv = ln_tmp.tile([P, CH], f32, tag="rcv")
        nc.vector.reciprocal(rcv[:], varp[:])
        rstd = ln_tmp.tile([P, CH], f32, tag="rstd")
        nc.scalar.activation(rstd[:], rcv[:], AF.Sqrt)
        rm = ln_tmp.tile([P, CH], f32, tag="rm")
        nc.vector.tensor_tensor(rm[:], rstd[:], mval[:], op=ALU.mult)
        for ft in range(FT):
            t0 = ln_tmp.tile([P, CH], f32, tag="t0")
            nc.vector.tensor_tensor(t0[:], src[:, ft, ts], rstd[:], op=ALU.mult)
            t1 = ln_tmp.tile([P, CH], f32, tag="t1")
            nc.vector.tensor_tensor(t1[:], t0[:], rm[:], op=ALU.subtract)
            nc.scalar.activation(out_b[:, ft, ts], t1[:], AF.Identity,
                                 bias=b_pm[:, ft:ft + 1], scale=g_pm[:, ft:ft + 1])


def build_nc():
    nc = bacc.Bacc("TRN2", target_bir_lowering=False)

    xT_d = nc.dram_tensor("xT", (F, T), f32, kind="ExternalInput")
    wposT_d = nc.dram_tensor("w_posT", (T, T), f32, kind="ExternalInput")
    wq_d = nc.dram_tensor("wq", (F, F), bf16, kind="ExternalInput")
    wk_d = nc.dram_tensor("wk", (F, F), bf16, kind="ExternalInput")
    wv_d = nc.dram_tensor("wv", (F, F), bf16, kind="ExternalInput")
    ow_d = nc.dram_tensor("ow", (F, F), bf16, kind="ExternalInput")
    w1_d = nc.dram_tensor("w1", (F, H), bf16, kind="ExternalInput")
    w2_d = nc.dram_tensor("w2", (H, F), bf16, kind="ExternalInput")
    wqb_d = nc.dram_tensor("wq_b", (F,), f32, kind="ExternalInput")
    wkb_d = nc.dram_tensor("wk_b", (F,), bf16, kind="ExternalInput")
    wvb_d = nc.dram_tensor("wv_b", (F,), bf16, kind="ExternalInput")
    outb_d = nc.dram_tensor("out_b", (F,), bf16, kind="ExternalInput")
    ln1g_d = nc.dram_tensor("ln1_g", (F,), f32, kind="ExternalInput")
    ln1b_d = nc.dram_tensor("ln1_b", (F,), f32, kind="ExternalInput")
    ln2g_d = nc.dram_tensor("ln2_g", (F,), f32, kind="ExternalInput")
    ln2b_d = nc.dram_tensor("ln2_b", (F,), f32, kind="ExternalInput")
    b1_d = nc.dram_tensor("mlp1_b", (H,), f32, kind="ExternalInput")
    b2_d = nc.dram_tensor("mlp2_b", (F,), f32, kind="ExternalInput")
    yT_d = nc.dram_tensor("yT", (F, T), f32, kind="ExternalOutput")

    with tile.TileContext(nc) as tc:
        with (
            tc.tile_pool(name="persist", bufs=1) as pp,
            tc.tile_pool(name="ln_tmp", bufs=2) as ln_tmp,
            tc.tile_pool(name="outstream", bufs=3) as outp,
            tc.tile_pool(name="psum", bufs=6, space="PSUM") as psum,
        ):
            # ---- persistent loads / constants
            xT = pp.tile([P, FT, T], f32)
            for ft in range(FT):
                nc.sync.dma_start(xT[:, ft, :], xT_d[ft * P:(ft + 1) * P, :])
            wq = pp.tile([P, FT, F], bf16)
            nc.sync.dma_start(wq[:], wq_d.rearrange("(a p) b -> p a b", p=P))
            wk = pp.tile([P, FT, F], bf16)
            nc.sync.dma_start(wk[:], wk_d.rearrange("(a p) b -> p a b", p=P))
            wv = pp.tile([P, FT, F], bf16)
            nc.sync.dma_start(wv[:], wv_d.rearrange("(a p) b -> p a b", p=P))
            ow = pp.tile([P, FT, F], bf16)
            nc.sync.dma_start(ow[:], ow_d.rearrange("(a p) b -> p a b", p=P))
            wqb = pp.tile([P, FT], f32)
            nc.sync.dma_start(wqb[:], wqb_d.rearrange("(a p) -> p a", p=P))
            wkb = pp.tile([1, F], bf16)
            nc.sync.dma_start(wkb[:], wkb_d[None, :])
            wvb = pp.tile([1, F], bf16)
            nc.sync.dma_start(wvb[:], wvb_d[None, :])
            outb = pp.tile([1, F], bf16)
            nc.sync.dma_start(outb[:], outb_d[None, :])
            ln1g = pp.tile([P, FT], f32)
            nc.sync.dma_start(ln1g[:], ln1g_d.rearrange("(a p) -> p a", p=P))
            ln1b = pp.tile([P, FT], f32)
            nc.sync.dma_start(ln1b[:], ln1b_d.rearrange("(a p) -> p a", p=P))
            ln2g = pp.tile([P, FT], f32)
            nc.sync.dma_start(ln2g[:], ln2g_d.rearrange("(a p) -> p a", p=P))
            ln2b = pp.tile([P, FT], f32)
            nc.sync.dma_start(ln2b[:], ln2b_d.rearrange("(a p) -> p a", p=P))
            b1 = pp.tile([P, HT], f32)
            nc.sync.dma_start(b1[:], b1_d.rearrange("(a p) -> p a", p=P))
            b2 = pp.tile([P, FT], f32)
            nc.sync.dma_start(b2[:], b2_d.rearrange("(a p) -> p a", p=P))
            ones = pp.tile([P, T], bf16)
            nc.vector.memset(ones[:], 1.0)

            yt = pp.tile([P, FT, T], bf16)    # sigma(Q)*num/den, T-layout
            outT = pp.tile([P, FT, T], f32)   # attn residual output, T-layout

            with tc.tile_pool(name="phaseA", bufs=1) as pa:
                xb = pa.tile([P, FT, T], bf16)
                sqb = pa.tile([P, FT, T], bf16)
                for ft in range(FT):
                    nc.vector.tensor_copy(xb[:, ft, :], xT[:, ft, :])
                    nc.scalar.square(sqb[:, ft, :], xT[:, ft, :])

                # ---- LN1 -> hTb (bf16, T-layout)
                hTb = pa.tile([P, FT, T], bf16)
                _emit_layernorm(nc, tc, psum, ln_tmp, xT, xb, sqb,
                                ln1g, ln1b, ones, hTb)

                # ---- exp_wT (bf16): stream w_posT tiles, exp on ACT
                expw = pa.tile([P, TT, T], bf16)
                with tc.tile_pool(name="wpos", bufs=2) as wpp:
                    for s in range(TT):
                        wp = wpp.tile([P, T], f32, tag="wp")
                        nc.sync.dma_start(wp[:], wposT_d[s * P:(s + 1) * P, :])
                        nc.scalar.activation(expw[:, s, :], wp[:], AF.Exp)

                # ---- K, V (N-layout [t, fo]) -> X = [ekV | ek] bf16 [P, TT, 2F]
                X = pa.tile([P, TT, 2 * F], bf16)
                for s in range(TT):
                    tsl = slice(s * P, (s + 1) * P)
                    kps = psum.tile([P, F], f32, tag="acc")
                    for ft in range(FT):
                        nc.tensor.matmul(kps[:], hTb[:, ft, tsl], wk[:, ft, :],
                                         start=(ft == 0), stop=False)
                    nc.tensor.matmul(kps[:], ones[0:1, :P], wkb[:],
                                     start=False, stop=True)
                    negmk = ln_tmp.tile([P, 1], f32, tag="negmk")
                    nc.vector.tensor_reduce(negmk[:], kps[:],
                                            axis=mybir.AxisListType.X,
                                            op=ALU.max, negate=True)
                    nc.scalar.activation(X[:, s, F:], kps[:], AF.Exp,
                                         bias=negmk[:], scale=1.0)
                    vps = psum.tile([P, F], f32, tag="acc")
                    for ft in range(FT):
                        nc.tensor.matmul(vps[:], hTb[:, ft, tsl], wv[:, ft, :],
                                         start=(ft == 0), stop=False)
                    nc.tensor.matmul(vps[:], ones[0:1, :P], wvb[:],
                                     start=False, stop=True)
                    nc.vector.tensor_tensor(X[:, s, :F], X[:, s, F:], vps[:],
                                            op=ALU.mult)

                # ---- Q (T-layout) -> sigQ (fp32)
                sigq = pa.tile([P, FT, T], f32)
                for fo in range(FT):
                    for c in range(NC):
                        ts = slice(c * CH, (c + 1) * CH)
                        qps = psum.tile([P, CH], f32, tag="acc")
                        for ft in range(FT):
                            nc.tensor.matmul(qps[:],
                                             wq[:, ft, fo * P:(fo + 1) * P],
                                             hTb[:, ft, ts],
                                             start=(ft == 0), stop=(ft == FT - 1))
                        nc.scalar.activation(sigq[:, fo, ts], qps[:], AF.Sigmoid,
                                             bias=wqb[:, fo:fo + 1], scale=1.0)

                # ---- num/den:  numT/denT[f, t] = X.T @ exp_wT  -> Yt
                with tc.tile_pool(name="ndtmp", bufs=2) as ndt:
                    for fo in range(FT):
                        for c in range(NC):
                            ts = slice(c * CH, (c + 1) * CH)
                            dps = psum.tile([P, CH], f32, tag="acc")
                            for s in range(TT):
                                nc.tensor.matmul(
                                    dps[:],
                                    X[:, s, F + fo * P:F + (fo + 1) * P],
                                    expw[:, s, ts],
                                    start=(s == 0), stop=(s == TT - 1))
                            rcden = ndt.tile([P, CH], f32, tag="rcden")
                            nc.vector.reciprocal(rcden[:], dps[:])
                            nps = psum.tile([P, CH], f32, tag="acc")
                            for s in range(TT):
                                nc.tensor.matmul(
                                    nps[:],
                                    X[:, s, fo * P:(fo + 1) * P],
                                    expw[:, s, ts],
                                    start=(s == 0), stop=(s == TT - 1))
                            t1 = ndt.tile([P, CH], f32, tag="t1")
                            nc.vector.tensor_tensor(t1[:], nps[:], rcden[:],
                                                    op=ALU.mult)
                            nc.vector.tensor_tensor(yt[:, fo, ts], t1[:],
                                                    sigq[:, fo, ts], op=ALU.mult)

            # ---- attn out (T-layout) + residual: outT = ow.T @ Yt + out_b + xT
            for g in range(FT):
                for c in range(NC):
                    ts = slice(c * CH, (c + 1) * CH)
                    aps = psum.tile([P, CH], f32, tag="acc")
                    for ft in range(FT):
                        nc.tensor.matmul(aps[:], ow[:, ft, g * P:(g + 1) * P],
                                         yt[:, ft, ts],
                                         start=(ft == 0), stop=False)
                    nc.tensor.matmul(aps[:], outb[0:1, g * P:(g + 1) * P],
                                     ones[0:1, :CH], start=False, stop=True)
                    nc.vector.scalar_tensor_tensor(outT[:, g, ts], aps[:], 1.0,
                                                   xT[:, g, ts],
                                                   op0=ALU.mult, op1=ALU.add)

            with tc.tile_pool(name="phaseB", bufs=1) as pb:
                # ---- LN2 -> mTb
                outb16 = pb.tile([P, FT, T], bf16)
                sq2b = pb.tile([P, FT, T], bf16)
                for ft in range(FT):
                    nc.vector.tensor_copy(outb16[:, ft, :], outT[:, ft, :])
                    nc.scalar.square(sq2b[:, ft, :], outT[:, ft, :])
                mTb = pb.tile([P, FT, T], bf16)
                _emit_layernorm(nc, tc, psum, ln_tmp, outT, outb16, sq2b,
                                ln2g, ln2b, ones, mTb)

                # ---- MLP
                w1 = pb.tile([P, FT, H], bf16)
                for ft in range(FT):
                    nc.sync.dma_start(
                        w1[:, ft, :], w1_d[ft * P:(ft + 1) * P, :])
                w2 = pb.tile([P, HT, F], bf16)
                for ht in range(HT):
                    nc.sync.dma_start(
                        w2[:, ht, :], w2_d[ht * P:(ht + 1) * P, :])

                m1 = pb.tile([P, HT, T], bf16)
                for ht in range(HT):
                    for c in range(NC):
                        ts = slice(c * CH, (c + 1) * CH)
                        mps = psum.tile([P, CH], f32, tag="acc")
                        for ft in range(FT):
                            nc.tensor.matmul(mps[:],
                                             w1[:, ft, ht * P:(ht + 1) * P],
                                             mTb[:, ft, ts],
                                             start=(ft == 0), stop=(ft == FT - 1))
                        nc.scalar.activation(m1[:, ht, ts], mps[:], AF.Gelu,
                                             bias=b1[:, ht:ht + 1], scale=1.0)

                for g in range(FT):
                    for c in range(NC):
                        ts = slice(c * CH, (c + 1) * CH)
                        fps = psum.tile([P, CH], f32, tag="acc")
                        for ht in range(HT):
                            nc.tensor.matmul(fps[:],
                                             w2[:, ht, g * P:(g + 1) * P],
                                             m1[:, ht, ts],
                                             start=(ht == 0), stop=(ht == HT - 1))
                        gt = outp.tile([P, CH], f32, tag="gt")
                        nc.scalar.activation(gt[:], fps[:], AF.Gelu,
                                             bias=b2[:, g:g + 1], scale=1.0)
                        fin = outp.tile([P, CH], f32, tag="fin")
                        nc.vector.tensor_tensor(fin[:], gt[:], outT[:, g, ts],
                                                op=ALU.add)
                        nc.sync.dma_start(yT_d[g * P:(g + 1) * P, ts], fin[:])
    nc.compile()
    return nc


@functools.lru_cache(maxsize=1)
def _get_nc():
    return build_nc()


def kernel(**inputs):
    x = np.asarray(inputs["x"], dtype=np.float32)           # [B, T, F]
    bf = lambda a: np.ascontiguousarray(np.asarray(a)).astype(ml_dtypes.bfloat16)
    fl = lambda a: np.ascontiguousarray(np.asarray(a), dtype=np.float32)

    shared = {
        "w_posT": fl(np.asarray(inputs["w_pos"]).T),
        "wq": bf(inputs["wq_w"]), "wk": bf(inputs["wk_w"]),
        "wv": bf(inputs["wv_w"]), "ow": bf(inputs["out_w"]),
        "w1": bf(inputs["mlp1_w"]), "w2": bf(inputs["mlp2_w"]),
        "wq_b": fl(inputs["wq_b"]), "wk_b": bf(inputs["wk_b"]),
        "wv_b": bf(inputs["wv_b"]), "out_b": bf(inputs["out_b"]),
        "ln1_g": fl(inputs["ln1_g"]), "ln1_b": fl(inputs["ln1_b"]),
        "ln2_g": fl(inputs["ln2_g"]), "ln2_b": fl(inputs["ln2_b"]),
        "mlp1_b": fl(inputs["mlp1_b"]), "mlp2_b": fl(inputs["mlp2_b"]),
    }
    in_maps = [
        {"xT": np.ascontiguousarray(x[c].T), **shared} for c in range(B)
    ]
    nc = _get_nc()
    res = run_bass_kernel_spmd(nc, in_maps, list(range(B)))
    out = np.stack([np.ascontiguousarray(res.results[c]["yT"].T)
                    for c in range(B)], axis=0)
    return out.astype(np.float32)


if __name__ == "__main__":
    rng = np.random.default_rng(0)
    fake = {
        "x": rng.standard_normal((B, T, F), dtype=np.float32),
        "wq_w": rng.standard_normal((F, F), dtype=np.float32) * 0.02,
        "wq_b": np.zeros(F, np.float32),
        "wk_w": rng.standard_normal((F, F), dtype=np.float32) * 0.02,
        "wk_b": np.zeros(F, np.float32),
        "wv_w": rng.standard_normal((F, F), dtype=np.float32) * 0.02,
        "wv_b": np.zeros(F, np.float32),
        "w_pos": rng.standard_normal((T, T), dtype=np.float32) * 0.05,
        "out_w": rng.standard_normal((F, F), dtype=np.float32) * 0.02,
        "out_b": np.zeros(F, np.float32),
        "ln1_g": np.ones(F, np.float32), "ln1_b": np.zeros(F, np.float32),
        "ln2_g": np.ones(F, np.float32), "ln2_b": np.zeros(F, np.float32),
        "mlp1_w": rng.standard_normal((F, H), dtype=np.float32) * 0.02,
        "mlp1_b": np.zeros(H, np.float32),
        "mlp2_w": rng.standard_normal((H, F), dtype=np.float32) * 0.02,
        "mlp2_b": np.zeros(F, np.float32),
    }
    y = kernel(**fake)
    print("kernel output:", y.shape, y.dtype, float(np.abs(y).max()))
